# revision 1
# baseline (speedup 1.0000x reference)
"""Multi-head self-attention TRN2 Bass kernel (8-core SPMD).

Problem: B=2, S=4096, E=512, H=8 heads (head dim 64), fp32.

Sharding: core c of 8 handles batch b=c//4 and head pair p=c%4
(columns 128p:128p+128 of the projection space) — each core owns
2 of the 16 (batch, head) score slabs. Inputs are sliced/transposed
on the host; each core returns a partial [S, E] output (its 2 heads'
contribution through Wo), summed per batch on the host.

Device algorithm (flash-style, no score materialization in HBM):
  Q^T[d,s], K^T[d,s] projected with d on partitions (both heads
  packed on partitions 0:64 / 64:128), V in natural [s,d] layout with
  a fused ones column; scores are computed TRANSPOSED S^T[k,q] =
  K^T.T @ Q^T so the PV contraction needs no transposes; exp runs
  without max-subtraction (scores ~N(0,1), fp32-safe); the ones
  column of V makes the PV matmul also produce the softmax
  denominators, which are moved to per-partition scalars with tiny
  PE transposes and applied after the output projection.

All matmul operands use float32r (e8m11 in an fp32 container): full
PE rate with ~1.2e-4 quantization error and exact fp32 PSUM
accumulation. The score matmuls of the two heads run concurrently as
64-row PE tiles (tile_position (0,0)/(64,0)). The PV matmuls trail
the score matmuls by 8 units (software pipeline) so the ScalarE exp
stream — the bottleneck engine at ~1 elem/lane/cycle — never stalls;
the Q/K/V projections are trickled into the early attention stream.

bk is dropped (softmax-invariant); bv@Wo and bo are added on the host
(exact: softmax rows sum to 1).
"""

import sys

sys.path.insert(0, "/opt/trn_rl_repo")

import numpy as np

import concourse.bass as bass
import concourse.bacc as bacc
import concourse.mybir as mybir
import concourse.tile as tile
from concourse.bass_utils import run_bass_kernel_spmd

F32 = mybir.dt.float32
F32R = mybir.dt.float32r
AF = mybir.ActivationFunctionType

N_CORES = 8
B, S, E = 2, 4096, 512
H, D = 8, 64


def round_f32r(x):
    """Round fp32 array to fp32r (e8m11: keep 11 mantissa bits, RNE)."""
    x = np.ascontiguousarray(x, dtype=np.float32)
    u = x.view(np.uint32).astype(np.uint64)
    r = (u + 0x7FF + ((u >> 12) & 1)) & 0xFFFFF000
    return r.astype(np.uint32).view(np.float32)


def build_mha(S=4096, E=512, QB=512, num_devices=8):
    EC = E // 128                # e-chunks for the projection contraction
    NQB = S // QB                # q blocks
    KT = S // 128                # k tiles
    MD = F32R

    nc = bacc.Bacc("TRN2", target_bir_lowering=False, debug=False,
                   num_devices=num_devices)

    xt = nc.dram_tensor("xt", [E, S], MD, kind="ExternalInput")
    wq = nc.dram_tensor("wq", [E, 128], MD, kind="ExternalInput")
    wk = nc.dram_tensor("wk", [E, 128], MD, kind="ExternalInput")
    wv = nc.dram_tensor("wv", [E, 256], MD, kind="ExternalInput")
    bq = nc.dram_tensor("bq", [128, 1], F32, kind="ExternalInput")
    ones = nc.dram_tensor("ones", [128, KT], MD, kind="ExternalInput")
    id2 = nc.dram_tensor("id2", [2, 2], F32, kind="ExternalInput")
    wo = nc.dram_tensor("wo", [128, E], MD, kind="ExternalInput")
    out = nc.dram_tensor("out", [S, E], F32, kind="ExternalOutput")

    with tile.TileContext(nc) as tc:
        with (
            tc.tile_pool(name="const", bufs=1) as const,
            tc.tile_pool(name="epool", bufs=10) as epool,
            tc.tile_pool(name="opool", bufs=2) as opool,
            tc.tile_pool(name="spsum", bufs=2, space="PSUM") as spsum,
            tc.tile_pool(name="apsum", bufs=2, space="PSUM") as apsum,
            tc.tile_pool(name="gpsum", bufs=2, space="PSUM") as gpsum,
        ):
            # DMA order matters (FIFO queue): small weight loads that gate
            # the first projections go first, then xt s-block 0, then the
            # rest of xt s-major.
            wq_sb = const.tile([128, EC, 128], MD, tag="wq")
            nc.sync.dma_start(wq_sb[:], wq.ap().rearrange("(c p) m -> p c m", p=128))
            wk_sb = const.tile([128, EC, 128], MD, tag="wk")
            nc.sync.dma_start(wk_sb[:], wk.ap().rearrange("(c p) m -> p c m", p=128))
            bq_sb = const.tile([128, 1], F32, tag="bq")
            nc.sync.dma_start(bq_sb[:], bq.ap())

            xt_sb = const.tile([128, EC, S], MD, tag="xt")
            xt_r = xt.ap().rearrange("(c p) s -> p c s", p=128)
            for c in range(EC):
                nc.sync.dma_start(xt_sb[:, c, bass.ts(0, QB)],
                                  xt_r[:, c, bass.ts(0, QB)])

            wv_sb = const.tile([128, EC, 256], MD, tag="wv")
            nc.sync.dma_start(wv_sb[:], wv.ap().rearrange("(c p) m -> p c m", p=128))
            wo_sb = const.tile([128, E], MD, tag="wo")
            nc.sync.dma_start(wo_sb[:], wo.ap())
            id2_sb = const.tile([2, 2], F32, tag="id2")
            nc.sync.dma_start(id2_sb[:], id2.ap())

            qt_sb = const.tile([128, S], MD, tag="qt")
            kt_sb = const.tile([128, S], MD, tag="kt")
            vn_sb = const.tile([128, KT, 130], MD, tag="vn")
            at_sb = const.tile([128, S], MD, tag="at")
            rcp = const.tile([128, S // 128, 2], F32, tag="rcp")

            nc.sync.dma_start(vn_sb[:, :, 64:65],
                              ones.ap().rearrange("p (t o) -> p t o", o=1))
            nc.sync.dma_start(vn_sb[:, :, 129:130],
                              ones.ap().rearrange("p (t o) -> p t o", o=1))

            for sb in range(1, NQB):
                for c in range(EC):
                    nc.sync.dma_start(xt_sb[:, c, bass.ts(sb, QB)],
                                      xt_r[:, c, bass.ts(sb, QB)])

            # ---- projection emitters (trickled into the attention stream)
            qk_state = {}

            def qk_mm(which, sb, c):
                scols = bass.ts(sb, QB)
                key = (which, sb)
                if c == 0:
                    qk_state[key] = gpsum.tile([128, QB], F32, tag="gp",
                                               name=f"p{which}_{sb}")
                w_sb = wq_sb if which == "q" else wk_sb
                nc.tensor.matmul(qk_state[key][:], w_sb[:, c, :],
                                 xt_sb[:, c, scols],
                                 start=(c == 0), stop=(c == EC - 1))
                if c == EC - 1:
                    p = qk_state.pop(key)
                    if which == "q":
                        nc.vector.tensor_scalar_add(qt_sb[:, scols], p[:],
                                                    bq_sb[:, 0:1])
                    else:
                        nc.vector.tensor_copy(kt_sb[:, scols], p[:])

            def v_chunk(t):
                tcols = bass.ts(t, 128)
                pv = gpsum.tile([128, 256], F32, tag="gp", name=f"pv_{t}")
                for c in range(EC):
                    nc.tensor.matmul(
                        pv[:], xt_sb[:, c, tcols], wv_sb[:, c, :],
                        start=(c == 0), stop=(c == EC - 1))
                nc.vector.tensor_copy(vn_sb[:, t, 0:64], pv[:, 0:64])
                nc.vector.tensor_copy(vn_sb[:, t, 65:129], pv[:, 64:128])

            for c in range(EC):
                qk_mm("k", 0, c)
            for c in range(EC):
                qk_mm("q", 0, c)
            for t in range(min(4, KT)):
                v_chunk(t)

            trickle = {}

            def put(u, fn):
                trickle.setdefault(u, []).append(fn)

            for b in range(1, NQB):
                for c in range(EC):
                    put(4 * (b - 1) + c, (lambda bb=b, cc=c: qk_mm("k", bb, cc)))
            for t in range(4, KT):
                put(t + 3, (lambda tt=t: v_chunk(tt)))
            for j in range(1, NQB):
                base = 28 if j == 1 else 35 + 4 * (j - 2)
                for c in range(EC):
                    put(base + c, (lambda jj=j, cc=c: qk_mm("q", jj, cc)))

            # ---- attention + output projection ----
            nt = QB // 128

            def epilogue_a(qb, pa0, pa1):
                qcols = bass.ts(qb, QB)
                nc.vector.tensor_copy(at_sb[0:64, qcols], pa0[0:64, :])
                nc.vector.tensor_copy(at_sb[64:128, qcols], pa1[0:64, :])
                drows = []
                for pa_ in (pa0, pa1):
                    drow = opool.tile([1, QB], F32, tag="drow")
                    nc.vector.tensor_copy(drow[:], pa_[64:65, :])
                    drows.append(drow)
                return drows

            def epilogue_b(qb, drows):
                for h, drow in enumerate(drows):
                    for j in range(nt):
                        t = qb * nt + j
                        ptr = gpsum.tile([128, 1], F32, tag="gp")
                        nc.tensor.transpose(ptr[:], drow[:, bass.ts(j, 128)],
                                            id2_sb[0:1, 0:1])
                        nc.vector.reciprocal(rcp[:, t, h:h + 1], ptr[:])

            def epilogue_c(qb, j):
                t = qb * nt + j
                tcols = bass.ts(t, 128)
                po0 = gpsum.tile([128, E], F32, tag="gp")
                nc.tensor.matmul(
                    po0[:], at_sb[0:64, tcols], wo_sb[0:64, :],
                    start=True, stop=True, tile_position=(0, 0))
                po1 = gpsum.tile([128, E], F32, tag="gp")
                nc.tensor.matmul(
                    po1[:], at_sb[64:128, tcols], wo_sb[64:128, :],
                    start=True, stop=True, tile_position=(64, 0))
                o0 = opool.tile([128, E], F32, tag="o0")
                nc.vector.tensor_scalar_mul(o0[:], po0[:], rcp[:, t, 0:1])
                o1 = opool.tile([128, E], F32, tag="o1")
                nc.vector.tensor_scalar_mul(o1[:], po1[:], rcp[:, t, 1:2])
                osb = opool.tile([128, E], F32, tag="osb")
                nc.vector.tensor_add(osb[:], o0[:], o1[:])
                nc.sync.dma_start(out.ap()[tcols, :], osb[:])

            units = [(qb, kt) for qb in range(NQB) for kt in range(KT)]
            pa = {}
            pending = []
            DEPTH = 8
            todo = []
            uidx = 0

            def flush_todo(limit):
                while todo and todo[0][0] <= limit:
                    todo.pop(0)[1]()

            def mm2_pair(pqb, pkt, pex):
                ppa0, ppa1 = pa[pqb]
                nc.tensor.matmul(
                    ppa0[:], vn_sb[:, pkt, 0:65], pex[:, 0:QB],
                    start=(pkt == 0), stop=(pkt == KT - 1))
                nc.tensor.matmul(
                    ppa1[:], vn_sb[:, pkt, 65:130], pex[:, QB:2 * QB],
                    start=(pkt == 0), stop=(pkt == KT - 1))
                if pkt == KT - 1:
                    drows = epilogue_a(pqb, ppa0, ppa1)
                    del pa[pqb]
                    todo.append((uidx + 1, (lambda q=pqb, d=drows:
                                            epilogue_b(q, d))))
                    for j in range(nt):
                        todo.append((uidx + 2 + 2 * j,
                                     (lambda q=pqb, jj=j: epilogue_c(q, jj))))

            for qb, kt in units:
                if kt == 0:
                    pa_t0 = apsum.tile([65, QB], F32, tag="pa", name=f"pa0_{qb}")
                    pa_t1 = apsum.tile([65, QB], F32, tag="pa", name=f"pa1_{qb}")
                    pa[qb] = (pa_t0, pa_t1)
                qcols = bass.ts(qb, QB)
                kcols = bass.ts(kt, 128)
                ps = spsum.tile([128, 2 * QB], F32, tag="ps")
                nc.tensor.matmul(
                    ps[:, 0:QB], kt_sb[0:64, kcols], qt_sb[0:64, qcols],
                    start=True, stop=True, tile_position=(0, 0))
                nc.tensor.matmul(
                    ps[:, QB:2 * QB], kt_sb[64:128, kcols], qt_sb[64:128, qcols],
                    start=True, stop=True, tile_position=(64, 0))
                if len(pending) >= DEPTH:
                    mm2_pair(*pending.pop(0))
                flush_todo(uidx)
                for fn in trickle.pop(uidx, ()):
                    fn()
                ex = epool.tile([128, 2 * QB], MD, tag="ex")
                nc.scalar.activation(ex[:], ps[:], AF.Exp, scale=0.125)
                pending.append((qb, kt, ex))
                uidx += 1

            for u in sorted(trickle):
                for fn in trickle.pop(u, ()):
                    fn()
            for item in pending:
                mm2_pair(*item)
                flush_todo(uidx)
                uidx += 1
            flush_todo(10 ** 9)

    nc.compile()
    return nc


_NC_CACHE = {}


def _get_nc():
    if "nc" not in _NC_CACHE:
        _NC_CACHE["nc"] = build_mha(S=S, E=E, num_devices=N_CORES)
    return _NC_CACHE["nc"]


def kernel(inputs, Wq, bq, Wk, bk, Wv, bv, Wo, bo):
    inputs = np.ascontiguousarray(inputs, dtype=np.float32)
    Wq = np.ascontiguousarray(Wq, dtype=np.float32)
    Wk = np.ascontiguousarray(Wk, dtype=np.float32)
    Wv = np.ascontiguousarray(Wv, dtype=np.float32)
    Wo = np.ascontiguousarray(Wo, dtype=np.float32)
    bq = np.ascontiguousarray(bq, dtype=np.float32)
    bv = np.ascontiguousarray(bv, dtype=np.float32)
    bo = np.ascontiguousarray(bo, dtype=np.float32)

    nc = _get_nc()
    pairs_per_batch = N_CORES // B
    kt_n = S // 128
    in_maps = []
    for c in range(N_CORES):
        b = c // pairs_per_batch
        p = c % pairs_per_batch
        cols = slice(128 * p, 128 * (p + 1))
        wv_pad = np.zeros((E, 256), dtype=np.float32)
        wv_pad[:, 0:128] = Wv[:, cols]
        in_maps.append({
            "xt": round_f32r(inputs[b].T),
            "wq": round_f32r(Wq[:, cols]),
            "wk": round_f32r(Wk[:, cols]),
            "wv": round_f32r(wv_pad),
            "bq": np.ascontiguousarray(bq[cols]).reshape(128, 1),
            "ones": np.ones((128, kt_n), dtype=np.float32),
            "id2": np.eye(2, dtype=np.float32),
            "wo": round_f32r(Wo[cols, :]),
        })

    res = run_bass_kernel_spmd(nc, in_maps, core_ids=list(range(N_CORES)))
    partials = [res.results[c]["out"] for c in range(N_CORES)]

    host_bias = (bv @ Wo + bo).astype(np.float32)
    outs = []
    for b in range(B):
        acc = partials[b * pairs_per_batch].astype(np.float32)
        for i in range(1, pairs_per_batch):
            acc = acc + partials[b * pairs_per_batch + i]
        outs.append(acc + host_bias)
    return np.stack(outs).astype(np.float32)
